# revision 13
# baseline (speedup 1.0000x reference)
"""Adaptive Computation Time step — Trainium2 Bass kernel.

Pure data parallel over batch B=8 on 8 NeuronCores (one batch row per core).

Per-core layout conventions (M=2048 tokens, H=1024):
  - "AT" small-tensor layout: SBUF [16, 128], token m at [m // 128, m % 128].
    DRAM [2048] <-> AT is a contiguous 512B-per-partition DMA.
  - "A" big-tile layout (forced by dma_gather): token m at partition m % 128,
    free chunk m // 128.  H tile is [128, 16, 1024].
  - "G" index layout (forced by dma_gather/dma_scatter_add): int16 index for
    token m at [m % 16, m // 16] of a [128, 128] tile (only partitions 0-15
    are used; the rest must be zeroed for the bounds assert).

Algorithm per core:
  rank   = cumsum(run) - 1                    (scan + 16x16 tri matmul)
  h      = x[clamp(rank,0)]                   (dma_gather, 4 x 512 rows)
  p      = sigmoid(h @ p_w + p_b) * run
  mc     = (acc_p + p < 0.99) & run ; me = run & ~mc
  update = p*mc + (1-acc_p)*me
  weighted_h_new = h*update + weighted_h      (fused DVE op, streamed)
  c2     = cumsum(mc);  dest = mc ? c2-1 : count_new + (m - c2)   (bijection)
  H[~mc rows] = pad_h  (copy_predicated), then scatter-add H -> h_packed
  (h_packed output buffer starts zeroed, each row written exactly once)
"""

import functools
import sys

sys.path.insert(0, "/opt/trn_rl_repo")

import numpy as np

B, M, H = 8, 2048, 1024
P, C = 128, 16  # partitions / chunks; M == P * C
TH = 0.99
NCORES = 8


@functools.lru_cache(maxsize=4)
def _build(updates_plus1: float):
    import concourse.bass as bass
    import concourse.tile as tile
    from concourse import bacc, mybir

    f32 = mybir.dt.float32
    i32 = mybir.dt.int32
    i16 = mybir.dt.int16
    u8 = mybir.dt.uint8
    Alu = mybir.AluOpType
    Act = mybir.ActivationFunctionType

    nc = bacc.Bacc("TRN2", target_bir_lowering=False, debug=False)

    # ---- DRAM parameters (per-core shard shapes) ----
    x = nc.dram_tensor("x", [M, H], f32, kind="ExternalInput")
    run8 = nc.dram_tensor("run8", [M], u8, kind="ExternalInput")
    accp = nc.dram_tensor("accp", [M], f32, kind="ExternalInput")
    wh = nc.dram_tensor("wh", [M, H], f32, kind="ExternalInput")
    rem = nc.dram_tensor("rem", [M], f32, kind="ExternalInput")
    exi = nc.dram_tensor("exi", [M], i32, kind="ExternalInput")
    pad128_d = nc.dram_tensor("pad128", [P, H], f32, kind="ExternalInput")
    pw128_d = nc.dram_tensor("pw128", [P, H], f32, kind="ExternalInput")
    pb_d = nc.dram_tensor("pb", [1], f32, kind="ExternalInput")
    iota_d = nc.dram_tensor("iota_c", [C, P], f32, kind="ExternalInput")
    us16_d = nc.dram_tensor("us16", [C, C], f32, kind="ExternalInput")
    on16_d = nc.dram_tensor("on16", [C, C], f32, kind="ExternalInput")
    id16_d = nc.dram_tensor("id16", [C, C], f32, kind="ExternalInput")
    id128_d = nc.dram_tensor("id128", [P, P], f32, kind="ExternalInput")

    hp = nc.dram_tensor("hp", [M, H], f32, kind="ExternalOutput")
    whn = nc.dram_tensor("whn", [M, H], f32, kind="ExternalOutput")
    accn = nc.dram_tensor("accn", [M], f32, kind="ExternalOutput")
    remn = nc.dram_tensor("remn", [M], f32, kind="ExternalOutput")
    runn = nc.dram_tensor("runn", [M], u8, kind="ExternalOutput")
    exn = nc.dram_tensor("exn", [M], i32, kind="ExternalOutput")

    def at_view(dram):  # DRAM [2048] -> [16,128] AT view
        return dram.ap().rearrange("(t p) -> t p", p=P)

    with tile.TileContext(nc) as tc:
        with (
            tc.tile_pool(name="const", bufs=1) as cst,
            tc.tile_pool(name="small", bufs=1) as sm,
            tc.tile_pool(name="idx", bufs=1) as idxp,
            tc.tile_pool(name="big", bufs=1) as big,
            tc.tile_pool(name="whs", bufs=3) as whs,
            tc.tile_pool(name="outs", bufs=3) as outs,
            tc.tile_pool(name="scr", bufs=2) as scr,
            tc.tile_pool(name="ps", bufs=2, space="PSUM") as ps,
            tc.tile_pool(name="ps2", bufs=2, space="PSUM") as ps2,
        ):
            # critical path first: run8 + the two matrices the rank
            # computation needs, on the sync HWDGE queue
            run8_t = sm.tile([C, P], u8)
            nc.sync.dma_start(out=run8_t[:], in_=at_view(run8))
            pb1 = cst.tile([1, 1], f32)
            nc.sync.dma_start(out=pb1[:], in_=pb_d.ap()[None, :])
            pb128 = cst.tile([P, 1], f32)
            nc.gpsimd.partition_broadcast(pb128[:], pb1[:])
            # mlp-library warm-up: a dummy 16-row gather forces the gpsimd
            # library load (~9us) to happen now, off the real gather path.
            # (partition_broadcast alone would make the pass pick the attn
            # library and the real gather would pay a second load.)
            warm_idx = idxp.tile([P, 1], i16)
            nc.vector.memset(warm_idx[:], 0)
            warm_out = scr.tile([P, 1, H], f32, tag="scr")
            nc.gpsimd.dma_gather(
                out_ap=warm_out[:], in_ap=x.ap(), idxs_ap=warm_idx[:],
                num_idxs=16, num_idxs_reg=16, elem_size=H,
            )
            id16 = cst.tile([C, C], f32)
            nc.sync.dma_start(out=id16[:], in_=id16_d.ap())
            us16 = cst.tile([C, C], f32)
            nc.sync.dma_start(out=us16[:], in_=us16_d.ap())
            # non-critical constants + small inputs on the scalar queue
            id128 = cst.tile([P, P], f32)
            nc.scalar.dma_start(out=id128[:], in_=id128_d.ap())
            on16 = cst.tile([C, C], f32)
            nc.scalar.dma_start(out=on16[:], in_=on16_d.ap())
            iota = cst.tile([C, P], f32)
            nc.scalar.dma_start(out=iota[:], in_=iota_d.ap())
            pw128 = cst.tile([P, H], f32)
            nc.scalar.dma_start(out=pw128[:], in_=pw128_d.ap())
            pad128 = cst.tile([P, H], f32)
            nc.scalar.dma_start(out=pad128[:], in_=pad128_d.ap())
            accp_t = sm.tile([C, P], f32)
            nc.scalar.dma_start(out=accp_t[:], in_=at_view(accp))
            rem_t = sm.tile([C, P], f32)
            nc.scalar.dma_start(out=rem_t[:], in_=at_view(rem))
            exi_t = sm.tile([C, P], i32)
            nc.scalar.dma_start(out=exi_t[:], in_=at_view(exi))
            runf = sm.tile([C, P], f32)
            nc.vector.tensor_copy(out=runf[:], in_=run8_t[:])
            exif = sm.tile([C, P], f32)
            nc.vector.tensor_copy(out=exif[:], in_=exi_t[:])

            z16 = sm.tile([C, P], f32)
            nc.vector.memset(z16[:], 0.0)

            # ---------------- cumsum #1 (rank from run) ----------------
            s1 = sm.tile([C, P], f32)
            nc.vector.tensor_tensor_scan(
                out=s1[:], data0=runf[:], data1=z16[:], initial=0.0,
                op0=Alu.add, op1=Alu.add,
            )
            ex1 = ps2.tile([C, 1], f32, space="PSUM", tag="tmisc")
            nc.tensor.matmul(out=ex1[:], lhsT=us16[:], rhs=s1[:, P - 1 : P],
                             start=True, stop=True)
            # rank = (s1 + excl) - 1 ; gather idx = max(rank, 0)
            c1 = sm.tile([C, P], f32)
            nc.vector.tensor_scalar(
                out=c1[:], in0=s1[:], scalar1=ex1[:], scalar2=1.0,
                op0=Alu.add, op1=Alu.subtract,
            )
            gidxf = sm.tile([C, P], f32)
            nc.vector.tensor_single_scalar(
                out=gidxf[:], in_=c1[:], scalar=0.0, op=Alu.max
            )

            # ---------------- AT -> G conversion for gather idx ----------------
            gidx16 = idxp.tile([P, P], i16)
            gview = gidx16[0:C, :].rearrange("q (t a) -> q a t", a=8)
            for a in range(8):
                pt = ps.tile([C, C], f32, space="PSUM", tag="blkT")
                nc.tensor.transpose(pt[:], gidxf[:, a * C : (a + 1) * C], id16[:])
                nc.vector.tensor_copy(out=gview[:, a, :], in_=pt[:])
            # the gather ucode runs on 8 Q7 cores; each reads its own
            # 16-partition replica of the index block (7 parallel copies)
            for r in range(1, 8):
                eng = nc.sync if r % 2 else nc.scalar
                eng.dma_start(out=gidx16[r * 16 : (r + 1) * 16, :],
                              in_=gidx16[0:16, :])

            # ---------------- big gather: H[token] = x[clamp(rank,0)] --------
            Ht = big.tile([P, C, H], f32)
            for g in range(8):
                nc.gpsimd.dma_gather(
                    out_ap=Ht[:, g * 2 : (g + 1) * 2, :],
                    in_ap=x.ap(),
                    idxs_ap=gidx16[:, g * 16 : (g + 1) * 16],
                    num_idxs=256,
                    num_idxs_reg=256,
                    elem_size=H,
                )

            # ---------------- weighted_h loads (independent) ----------------
            wh_tiles = []
            for t in range(C):
                wt = whs.tile([P, H], f32, tag="wht")
                nc.sync.dma_start(out=wt[:], in_=wh.ap()[t * P : (t + 1) * P, :])
                wh_tiles.append(wt)

            # ---------------- p-dot per chunk ----------------
            pdot = sm.tile([P, C], f32)
            for t in range(C):
                sc = scr.tile([P, H], f32, tag="scr")
                nc.vector.scalar_tensor_tensor(
                    out=sc[:], in0=Ht[:, t, :], scalar=1.0, in1=pw128[:],
                    op0=Alu.mult, op1=Alu.mult,
                    accum_out=pdot[:, t : t + 1],
                )

            sig_a = sm.tile([P, C], f32)
            nc.scalar.activation(sig_a[:], pdot[:], Act.Sigmoid,
                                 bias=pb128[:], scale=1.0)
            # transpose sigmoid result A -> AT
            pts = ps2.tile([C, P], f32, space="PSUM", tag="tmisc")
            nc.tensor.transpose(pts[:], sig_a[:], id128[:])
            sig = sm.tile([C, P], f32)
            nc.scalar.copy(out=sig[:], in_=pts[:])

            # ---------------- halting math (AT layout) ----------------
            p_at = sm.tile([C, P], f32)
            nc.vector.tensor_mul(out=p_at[:], in0=sig[:], in1=runf[:])
            tmp = sm.tile([C, P], f32)
            nc.vector.tensor_add(out=tmp[:], in0=accp_t[:], in1=p_at[:])
            mcf = sm.tile([C, P], f32)
            nc.vector.tensor_single_scalar(out=mcf[:], in_=tmp[:], scalar=TH,
                                           op=Alu.is_lt)
            mc = sm.tile([C, P], f32)
            nc.vector.tensor_mul(out=mc[:], in0=mcf[:], in1=runf[:])
            me = sm.tile([C, P], f32)
            nc.vector.tensor_sub(out=me[:], in0=runf[:], in1=mc[:])
            u1 = sm.tile([C, P], f32)
            nc.vector.tensor_mul(out=u1[:], in0=p_at[:], in1=mc[:])
            t2 = sm.tile([C, P], f32)
            nc.vector.tensor_mul(out=t2[:], in0=accp_t[:], in1=me[:])
            u2 = sm.tile([C, P], f32)
            nc.vector.tensor_sub(out=u2[:], in0=me[:], in1=t2[:])
            upd = sm.tile([C, P], f32)
            nc.vector.tensor_add(out=upd[:], in0=u1[:], in1=u2[:])
            accn_t = sm.tile([C, P], f32)
            nc.vector.tensor_add(out=accn_t[:], in0=accp_t[:], in1=u1[:])
            remn_t = sm.tile([C, P], f32)
            nc.vector.tensor_add(out=remn_t[:], in0=rem_t[:], in1=u2[:])
            exnf = sm.tile([C, P], f32)
            nc.vector.scalar_tensor_tensor(
                out=exnf[:], in0=me[:], scalar=updates_plus1, in1=exif[:],
                op0=Alu.mult, op1=Alu.add,
            )
            inv = sm.tile([C, P], f32)
            nc.vector.tensor_single_scalar(out=inv[:], in_=mc[:], scalar=0.0,
                                           op=Alu.is_equal)

            # ---------------- cumsum #2 (pack positions from mc) -------------
            s2 = sm.tile([C, P], f32)
            nc.vector.tensor_tensor_scan(
                out=s2[:], data0=mc[:], data1=z16[:], initial=0.0,
                op0=Alu.add, op1=Alu.add,
            )
            ex2 = ps2.tile([C, 1], f32, space="PSUM", tag="tmisc")
            nc.tensor.matmul(out=ex2[:], lhsT=us16[:], rhs=s2[:, P - 1 : P],
                             start=True, stop=True)
            cnt = ps2.tile([C, 1], f32, space="PSUM", tag="tmisc")
            nc.tensor.matmul(out=cnt[:], lhsT=on16[:], rhs=s2[:, P - 1 : P],
                             start=True, stop=True)
            c2 = sm.tile([C, P], f32)
            nc.vector.tensor_scalar(
                out=c2[:], in0=s2[:], scalar1=ex2[:], scalar2=1.0,
                op0=Alu.add, op1=Alu.subtract,
            )
            # tail slot = (iota - c2) + count_new - 1
            tl0 = sm.tile([C, P], f32)
            nc.vector.tensor_sub(out=tl0[:], in0=iota[:], in1=c2[:])
            dest = sm.tile([C, P], f32)
            nc.vector.tensor_scalar(
                out=dest[:], in0=tl0[:], scalar1=cnt[:], scalar2=1.0,
                op0=Alu.add, op1=Alu.subtract,
            )
            mc8 = sm.tile([C, P], u8)
            nc.vector.tensor_copy(out=mc8[:], in_=mc[:])
            nc.vector.copy_predicated(out=dest[:], mask=mc8[:], data=c2[:])

            # ---------------- AT -> G conversion for scatter idx -------------
            didx16 = idxp.tile([P, P], i16)
            dview = didx16[0:C, :].rearrange("q (t a) -> q a t", a=8)
            for a in range(8):
                pt2 = ps.tile([C, C], f32, space="PSUM", tag="blkT")
                nc.tensor.transpose(pt2[:], dest[:, a * C : (a + 1) * C], id16[:])
                nc.vector.tensor_copy(out=dview[:, a, :], in_=pt2[:])
            for r in range(1, 8):
                eng = nc.sync if r % 2 else nc.scalar
                eng.dma_start(out=didx16[r * 16 : (r + 1) * 16, :],
                              in_=didx16[0:16, :])

            # ---------------- update / pad-mask to A layout ----------------
            ptu = ps2.tile([P, C], f32, space="PSUM", tag="tmisc")
            nc.tensor.transpose(ptu[:], upd[:], id16[:])
            upd_a = sm.tile([P, C], f32)
            nc.scalar.copy(out=upd_a[:], in_=ptu[:])
            pti = ps2.tile([P, C], f32, space="PSUM", tag="tmisc")
            nc.tensor.transpose(pti[:], inv[:], id16[:])
            inv_a = sm.tile([P, C], u8)
            nc.vector.tensor_copy(out=inv_a[:], in_=pti[:])

            # ---------------- FMA + stores, pad-predicate, scatter -----------
            for t in range(C):
                ot = outs.tile([P, H], f32, tag="outt")
                nc.vector.scalar_tensor_tensor(
                    out=ot[:], in0=Ht[:, t, :], scalar=upd_a[:, t : t + 1],
                    in1=wh_tiles[t][:], op0=Alu.mult, op1=Alu.add,
                )
                nc.scalar.dma_start(out=whn.ap()[t * P : (t + 1) * P, :], in_=ot[:])
                nc.vector.copy_predicated(
                    out=Ht[:, t, :],
                    mask=inv_a[:, t : t + 1].to_broadcast([P, H]),
                    data=pad128[:],
                )

            for g in range(8):
                nc.gpsimd.dma_scatter_add(
                    out_ap=hp.ap(),
                    in_ap=Ht[:, g * 2 : (g + 1) * 2, :],
                    idxs_ap=didx16[:, g * 16 : (g + 1) * 16],
                    num_idxs=256,
                    num_idxs_reg=256,
                    elem_size=H,
                )
                # every call's out_ap covers all of hp, but the destination
                # rows are disjoint by construction (dest is a bijection) —
                # drop the spurious WAW chain so the scatters pipeline; the
                # kernel-tail gpsimd drain still awaits the SWDGE queues
                tc.dep_state.clear_tensor_accesses(hp.name)

            # ---------------- small outputs ----------------
            nc.scalar.dma_start(out=at_view(accn), in_=accn_t[:])
            nc.scalar.dma_start(out=at_view(remn), in_=remn_t[:])
            exn_t = sm.tile([C, P], i32)
            nc.vector.tensor_copy(out=exn_t[:], in_=exnf[:])
            nc.scalar.dma_start(out=at_view(exn), in_=exn_t[:])
            nc.scalar.dma_start(out=at_view(runn), in_=mc8[:])

    nc.compile()
    return nc


def _consts():
    iota = np.arange(M, dtype=np.float32).reshape(C, P)
    us16 = np.triu(np.ones((C, C), dtype=np.float32), 1)  # [k,i]=1 iff k<i
    on16 = np.ones((C, C), dtype=np.float32)
    id16 = np.eye(C, dtype=np.float32)
    id128 = np.eye(P, dtype=np.float32)
    return iota, us16, on16, id16, id128


def make_in_maps(x, run, acc_p, weighted_h, remainders, exit_, pad_h, p_w, p_b):
    iota, us16, on16, id16, id128 = _consts()
    pad128 = np.broadcast_to(
        np.asarray(pad_h, dtype=np.float32).reshape(1, H), (P, H)).copy()
    pw128 = np.broadcast_to(
        np.asarray(p_w, dtype=np.float32).reshape(1, H), (P, H)).copy()
    pb1 = np.asarray(p_b, dtype=np.float32).reshape(1)
    in_maps = []
    for b in range(NCORES):
        in_maps.append(
            {
                "x": np.ascontiguousarray(x[b], dtype=np.float32),
                "run8": np.ascontiguousarray(run[b]).astype(np.uint8),
                "accp": np.ascontiguousarray(acc_p[b]).reshape(M).astype(np.float32),
                "wh": np.ascontiguousarray(weighted_h[b], dtype=np.float32),
                "rem": np.ascontiguousarray(remainders[b]).reshape(M).astype(np.float32),
                "exi": np.ascontiguousarray(exit_[b]).reshape(M).astype(np.int32),
                "pad128": pad128,
                "pw128": pw128,
                "pb": pb1,
                "iota_c": iota,
                "us16": us16,
                "on16": on16,
                "id16": id16,
                "id128": id128,
            }
        )
    return in_maps


def kernel(x, run, acc_p, weighted_h, remainders, exit_, updates, pad_h, p_w, p_b,
           _want_results_obj=False, _trace=False):
    from concourse.bass_utils import run_bass_kernel_spmd

    x = np.asarray(x)
    run = np.asarray(run)
    acc_p = np.asarray(acc_p)
    weighted_h = np.asarray(weighted_h)
    remainders = np.asarray(remainders)
    exit_ = np.asarray(exit_)

    nc = _build(float(np.asarray(updates)) + 1.0)
    in_maps = make_in_maps(x, run, acc_p, weighted_h, remainders, exit_,
                           pad_h, p_w, p_b)
    res = run_bass_kernel_spmd(nc, in_maps, core_ids=list(range(NCORES)),
                               trace=_trace)

    h_packed = np.stack([res.results[b]["hp"] for b in range(NCORES)])
    whn = np.stack([res.results[b]["whn"] for b in range(NCORES)])
    accn = np.stack([res.results[b]["accn"] for b in range(NCORES)]).reshape(B, M, 1)
    remn = np.stack([res.results[b]["remn"] for b in range(NCORES)]).reshape(B, M, 1)
    runn = np.stack([res.results[b]["runn"] for b in range(NCORES)]).astype(bool)
    exn = np.stack([res.results[b]["exn"] for b in range(NCORES)]).reshape(B, M, 1)
    out = (h_packed, whn, accn, remn, runn, exn.astype(np.int32))
    if _want_results_obj:
        return out, res
    return out


# revision 14
# speedup vs baseline: 1.0982x; 1.0982x over previous
"""Adaptive Computation Time step — Trainium2 Bass kernel.

Pure data parallel over batch B=8 on 8 NeuronCores (one batch row per core).

Per-core layout conventions (M=2048 tokens, H=1024):
  - "AT" small-tensor layout: SBUF [16, 128], token m at [m // 128, m % 128].
    DRAM [2048] <-> AT is a contiguous 512B-per-partition DMA.
  - "A" big-tile layout (forced by dma_gather): token m at partition m % 128,
    free chunk m // 128.  H tile is [128, 16, 1024].
  - "G" index layout (forced by dma_gather/dma_scatter_add): int16 index for
    token m at [m % 16, m // 16] of a [128, 128] tile (only partitions 0-15
    are used; the rest must be zeroed for the bounds assert).

Algorithm per core:
  rank   = cumsum(run) - 1                    (scan + 16x16 tri matmul)
  h      = x[clamp(rank,0)]                   (dma_gather, 4 x 512 rows)
  p      = sigmoid(h @ p_w + p_b) * run
  mc     = (acc_p + p < 0.99) & run ; me = run & ~mc
  update = p*mc + (1-acc_p)*me
  weighted_h_new = h*update + weighted_h      (fused DVE op, streamed)
  c2     = cumsum(mc);  dest = mc ? c2-1 : count_new + (m - c2)   (bijection)
  H[~mc rows] = pad_h  (copy_predicated), then scatter-add H -> h_packed
  (h_packed output buffer starts zeroed, each row written exactly once)
"""

import functools
import sys

sys.path.insert(0, "/opt/trn_rl_repo")

import numpy as np

B, M, H = 8, 2048, 1024
P, C = 128, 16  # partitions / chunks; M == P * C
TH = 0.99
NCORES = 8


@functools.lru_cache(maxsize=4)
def _build(updates_plus1: float):
    import concourse.bass as bass
    import concourse.tile as tile
    from concourse import bacc, mybir

    f32 = mybir.dt.float32
    i32 = mybir.dt.int32
    i16 = mybir.dt.int16
    u8 = mybir.dt.uint8
    Alu = mybir.AluOpType
    Act = mybir.ActivationFunctionType

    nc = bacc.Bacc("TRN2", target_bir_lowering=False, debug=False)

    # ---- DRAM parameters (per-core shard shapes) ----
    x = nc.dram_tensor("x", [M, H], f32, kind="ExternalInput")
    run8 = nc.dram_tensor("run8", [M], u8, kind="ExternalInput")
    accp = nc.dram_tensor("accp", [M], f32, kind="ExternalInput")
    wh = nc.dram_tensor("wh", [M, H], f32, kind="ExternalInput")
    rem = nc.dram_tensor("rem", [M], f32, kind="ExternalInput")
    exi = nc.dram_tensor("exi", [M], i32, kind="ExternalInput")
    pad128_d = nc.dram_tensor("pad128", [P, H], f32, kind="ExternalInput")
    pw128_d = nc.dram_tensor("pw128", [P, H], f32, kind="ExternalInput")
    pb_d = nc.dram_tensor("pb", [1], f32, kind="ExternalInput")
    iota_d = nc.dram_tensor("iota_c", [C, P], f32, kind="ExternalInput")
    us16_d = nc.dram_tensor("us16", [C, C], f32, kind="ExternalInput")
    on16_d = nc.dram_tensor("on16", [C, C], f32, kind="ExternalInput")
    id16_d = nc.dram_tensor("id16", [C, C], f32, kind="ExternalInput")
    id128_d = nc.dram_tensor("id128", [P, P], f32, kind="ExternalInput")

    hp = nc.dram_tensor("hp", [M, H], f32, kind="ExternalOutput")
    whn = nc.dram_tensor("whn", [M, H], f32, kind="ExternalOutput")
    accn = nc.dram_tensor("accn", [M], f32, kind="ExternalOutput")
    remn = nc.dram_tensor("remn", [M], f32, kind="ExternalOutput")
    runn = nc.dram_tensor("runn", [M], u8, kind="ExternalOutput")
    exn = nc.dram_tensor("exn", [M], i32, kind="ExternalOutput")

    def at_view(dram):  # DRAM [2048] -> [16,128] AT view
        return dram.ap().rearrange("(t p) -> t p", p=P)

    with tile.TileContext(nc) as tc:
        with (
            tc.tile_pool(name="const", bufs=1) as cst,
            tc.tile_pool(name="small", bufs=1) as sm,
            tc.tile_pool(name="idx", bufs=1) as idxp,
            tc.tile_pool(name="big", bufs=1) as big,
            tc.tile_pool(name="whs", bufs=8) as whs,
            tc.tile_pool(name="outs", bufs=4) as outs,
            tc.tile_pool(name="scr", bufs=2) as scr,
            tc.tile_pool(name="ps", bufs=2, space="PSUM") as ps,
            tc.tile_pool(name="ps2", bufs=2, space="PSUM") as ps2,
        ):
            # critical path first: run8 + the two matrices the rank
            # computation needs, on the sync HWDGE queue
            run8_t = sm.tile([C, P], u8)
            nc.sync.dma_start(out=run8_t[:], in_=at_view(run8))
            pb1 = cst.tile([1, 1], f32)
            nc.sync.dma_start(out=pb1[:], in_=pb_d.ap()[None, :])
            pb128 = cst.tile([P, 1], f32)
            nc.gpsimd.partition_broadcast(pb128[:], pb1[:])
            # mlp-library warm-up: a dummy 16-row gather forces the gpsimd
            # library load (~9us) to happen now, off the real gather path.
            # (partition_broadcast alone would make the pass pick the attn
            # library and the real gather would pay a second load.)
            warm_idx = idxp.tile([P, 1], i16)
            nc.vector.memset(warm_idx[:], 0)
            warm_out = scr.tile([P, 1, H], f32, tag="scr")
            nc.gpsimd.dma_gather(
                out_ap=warm_out[:], in_ap=x.ap(), idxs_ap=warm_idx[:],
                num_idxs=16, num_idxs_reg=16, elem_size=H,
            )
            id16 = cst.tile([C, C], f32)
            nc.sync.dma_start(out=id16[:], in_=id16_d.ap())
            us16 = cst.tile([C, C], f32)
            nc.sync.dma_start(out=us16[:], in_=us16_d.ap())
            # non-critical constants + small inputs on the scalar queue
            id128 = cst.tile([P, P], f32)
            nc.scalar.dma_start(out=id128[:], in_=id128_d.ap())
            on16 = cst.tile([C, C], f32)
            nc.scalar.dma_start(out=on16[:], in_=on16_d.ap())
            iota = cst.tile([C, P], f32)
            nc.scalar.dma_start(out=iota[:], in_=iota_d.ap())
            pw128 = cst.tile([P, H], f32)
            nc.scalar.dma_start(out=pw128[:], in_=pw128_d.ap())
            pad128 = cst.tile([P, H], f32)
            nc.scalar.dma_start(out=pad128[:], in_=pad128_d.ap())
            accp_t = sm.tile([C, P], f32)
            nc.scalar.dma_start(out=accp_t[:], in_=at_view(accp))
            rem_t = sm.tile([C, P], f32)
            nc.scalar.dma_start(out=rem_t[:], in_=at_view(rem))
            exi_t = sm.tile([C, P], i32)
            nc.scalar.dma_start(out=exi_t[:], in_=at_view(exi))
            runf = sm.tile([C, P], f32)
            nc.vector.tensor_copy(out=runf[:], in_=run8_t[:])
            exif = sm.tile([C, P], f32)
            nc.vector.tensor_copy(out=exif[:], in_=exi_t[:])

            z16 = sm.tile([C, P], f32)
            nc.vector.memset(z16[:], 0.0)

            # ---------------- cumsum #1 (rank from run) ----------------
            s1 = sm.tile([C, P], f32)
            nc.vector.tensor_tensor_scan(
                out=s1[:], data0=runf[:], data1=z16[:], initial=0.0,
                op0=Alu.add, op1=Alu.add,
            )
            ex1 = ps2.tile([C, 1], f32, space="PSUM", tag="tmisc")
            nc.tensor.matmul(out=ex1[:], lhsT=us16[:], rhs=s1[:, P - 1 : P],
                             start=True, stop=True)
            # rank = (s1 + excl) - 1 ; gather idx = max(rank, 0)
            c1 = sm.tile([C, P], f32)
            nc.vector.tensor_scalar(
                out=c1[:], in0=s1[:], scalar1=ex1[:], scalar2=1.0,
                op0=Alu.add, op1=Alu.subtract,
            )
            gidxf = sm.tile([C, P], f32)
            nc.vector.tensor_single_scalar(
                out=gidxf[:], in_=c1[:], scalar=0.0, op=Alu.max
            )

            # ---------------- AT -> G conversion for gather idx ----------------
            gidx16 = idxp.tile([P, P], i16)
            gview = gidx16[0:C, :].rearrange("q (t a) -> q a t", a=8)
            for a in range(8):
                pt = ps.tile([C, C], f32, space="PSUM", tag="blkT")
                nc.tensor.transpose(pt[:], gidxf[:, a * C : (a + 1) * C], id16[:])
                nc.vector.tensor_copy(out=gview[:, a, :], in_=pt[:])
            # the gather ucode runs on 8 Q7 cores; each reads its own
            # 16-partition replica of the index block. SWDGE (gpsimd) is idle
            # here, and the HWDGE queues carry bulk loads that would delay
            # these small copies.
            for r in range(1, 8):
                nc.gpsimd.dma_start(out=gidx16[r * 16 : (r + 1) * 16, :],
                                    in_=gidx16[0:16, :])

            # ---------------- big gather: H[token] = x[clamp(rank,0)] --------
            Ht = big.tile([P, C, H], f32)
            for g in range(8):
                nc.gpsimd.dma_gather(
                    out_ap=Ht[:, g * 2 : (g + 1) * 2, :],
                    in_ap=x.ap(),
                    idxs_ap=gidx16[:, g * 16 : (g + 1) * 16],
                    num_idxs=256,
                    num_idxs_reg=256,
                    elem_size=H,
                )

            # ---------------- weighted_h loads (independent) ----------------
            wh_tiles = []
            for t in range(C):
                wt = whs.tile([P, H], f32, tag="wht")
                nc.sync.dma_start(out=wt[:], in_=wh.ap()[t * P : (t + 1) * P, :])
                wh_tiles.append(wt)

            # ---------------- p-dot per chunk ----------------
            pdot = sm.tile([P, C], f32)
            for t in range(C):
                sc = scr.tile([P, H], f32, tag="scr")
                nc.vector.scalar_tensor_tensor(
                    out=sc[:], in0=Ht[:, t, :], scalar=1.0, in1=pw128[:],
                    op0=Alu.mult, op1=Alu.mult,
                    accum_out=pdot[:, t : t + 1],
                )

            sig_a = sm.tile([P, C], f32)
            nc.scalar.activation(sig_a[:], pdot[:], Act.Sigmoid,
                                 bias=pb128[:], scale=1.0)
            # transpose sigmoid result A -> AT
            pts = ps2.tile([C, P], f32, space="PSUM", tag="tmisc")
            nc.tensor.transpose(pts[:], sig_a[:], id128[:])
            sig = sm.tile([C, P], f32)
            nc.scalar.copy(out=sig[:], in_=pts[:])

            # ---------------- halting math (AT layout) ----------------
            p_at = sm.tile([C, P], f32)
            nc.vector.tensor_mul(out=p_at[:], in0=sig[:], in1=runf[:])
            tmp = sm.tile([C, P], f32)
            nc.vector.tensor_add(out=tmp[:], in0=accp_t[:], in1=p_at[:])
            mcf = sm.tile([C, P], f32)
            nc.vector.tensor_single_scalar(out=mcf[:], in_=tmp[:], scalar=TH,
                                           op=Alu.is_lt)
            mc = sm.tile([C, P], f32)
            nc.vector.tensor_mul(out=mc[:], in0=mcf[:], in1=runf[:])
            me = sm.tile([C, P], f32)
            nc.vector.tensor_sub(out=me[:], in0=runf[:], in1=mc[:])
            u1 = sm.tile([C, P], f32)
            nc.vector.tensor_mul(out=u1[:], in0=p_at[:], in1=mc[:])
            t2 = sm.tile([C, P], f32)
            nc.vector.tensor_mul(out=t2[:], in0=accp_t[:], in1=me[:])
            u2 = sm.tile([C, P], f32)
            nc.vector.tensor_sub(out=u2[:], in0=me[:], in1=t2[:])
            upd = sm.tile([C, P], f32)
            nc.vector.tensor_add(out=upd[:], in0=u1[:], in1=u2[:])
            accn_t = sm.tile([C, P], f32)
            nc.vector.tensor_add(out=accn_t[:], in0=accp_t[:], in1=u1[:])
            remn_t = sm.tile([C, P], f32)
            nc.vector.tensor_add(out=remn_t[:], in0=rem_t[:], in1=u2[:])
            exnf = sm.tile([C, P], f32)
            nc.vector.scalar_tensor_tensor(
                out=exnf[:], in0=me[:], scalar=updates_plus1, in1=exif[:],
                op0=Alu.mult, op1=Alu.add,
            )
            inv = sm.tile([C, P], f32)
            nc.vector.tensor_single_scalar(out=inv[:], in_=mc[:], scalar=0.0,
                                           op=Alu.is_equal)

            # ---------------- cumsum #2 (pack positions from mc) -------------
            s2 = sm.tile([C, P], f32)
            nc.vector.tensor_tensor_scan(
                out=s2[:], data0=mc[:], data1=z16[:], initial=0.0,
                op0=Alu.add, op1=Alu.add,
            )
            ex2 = ps2.tile([C, 1], f32, space="PSUM", tag="tmisc")
            nc.tensor.matmul(out=ex2[:], lhsT=us16[:], rhs=s2[:, P - 1 : P],
                             start=True, stop=True)
            cnt = ps2.tile([C, 1], f32, space="PSUM", tag="tmisc")
            nc.tensor.matmul(out=cnt[:], lhsT=on16[:], rhs=s2[:, P - 1 : P],
                             start=True, stop=True)
            c2 = sm.tile([C, P], f32)
            nc.vector.tensor_scalar(
                out=c2[:], in0=s2[:], scalar1=ex2[:], scalar2=1.0,
                op0=Alu.add, op1=Alu.subtract,
            )
            # tail slot = (iota - c2) + count_new - 1
            tl0 = sm.tile([C, P], f32)
            nc.vector.tensor_sub(out=tl0[:], in0=iota[:], in1=c2[:])
            dest = sm.tile([C, P], f32)
            nc.vector.tensor_scalar(
                out=dest[:], in0=tl0[:], scalar1=cnt[:], scalar2=1.0,
                op0=Alu.add, op1=Alu.subtract,
            )
            mc8 = sm.tile([C, P], u8)
            nc.vector.tensor_copy(out=mc8[:], in_=mc[:])
            nc.vector.copy_predicated(out=dest[:], mask=mc8[:], data=c2[:])

            # ---------------- AT -> G conversion for scatter idx -------------
            didx16 = idxp.tile([P, P], i16)
            dview = didx16[0:C, :].rearrange("q (t a) -> q a t", a=8)
            for a in range(8):
                pt2 = ps.tile([C, C], f32, space="PSUM", tag="blkT")
                nc.tensor.transpose(pt2[:], dest[:, a * C : (a + 1) * C], id16[:])
                nc.vector.tensor_copy(out=dview[:, a, :], in_=pt2[:])
            for r in range(1, 8):
                nc.gpsimd.dma_start(out=didx16[r * 16 : (r + 1) * 16, :],
                                    in_=didx16[0:16, :])

            # ---------------- update / pad-mask to A layout ----------------
            ptu = ps2.tile([P, C], f32, space="PSUM", tag="tmisc")
            nc.tensor.transpose(ptu[:], upd[:], id16[:])
            upd_a = sm.tile([P, C], f32)
            nc.scalar.copy(out=upd_a[:], in_=ptu[:])
            pti = ps2.tile([P, C], f32, space="PSUM", tag="tmisc")
            nc.tensor.transpose(pti[:], inv[:], id16[:])
            inv_a = sm.tile([P, C], u8)
            nc.vector.tensor_copy(out=inv_a[:], in_=pti[:])

            # ---------------- FMA + stores, pad-predicate, scatter -----------
            for t in range(C):
                ot = outs.tile([P, H], f32, tag="outt")
                nc.vector.scalar_tensor_tensor(
                    out=ot[:], in0=Ht[:, t, :], scalar=upd_a[:, t : t + 1],
                    in1=wh_tiles[t][:], op0=Alu.mult, op1=Alu.add,
                )
                nc.scalar.dma_start(out=whn.ap()[t * P : (t + 1) * P, :], in_=ot[:])
                nc.vector.copy_predicated(
                    out=Ht[:, t, :],
                    mask=inv_a[:, t : t + 1].to_broadcast([P, H]),
                    data=pad128[:],
                )

            for g in range(8):
                nc.gpsimd.dma_scatter_add(
                    out_ap=hp.ap(),
                    in_ap=Ht[:, g * 2 : (g + 1) * 2, :],
                    idxs_ap=didx16[:, g * 16 : (g + 1) * 16],
                    num_idxs=256,
                    num_idxs_reg=256,
                    elem_size=H,
                )
                # every call's out_ap covers all of hp, but the destination
                # rows are disjoint by construction (dest is a bijection) —
                # drop the spurious WAW chain so the scatters pipeline; the
                # kernel-tail gpsimd drain still awaits the SWDGE queues
                tc.dep_state.clear_tensor_accesses(hp.name)

            # ---------------- small outputs ----------------
            nc.scalar.dma_start(out=at_view(accn), in_=accn_t[:])
            nc.scalar.dma_start(out=at_view(remn), in_=remn_t[:])
            exn_t = sm.tile([C, P], i32)
            nc.vector.tensor_copy(out=exn_t[:], in_=exnf[:])
            nc.scalar.dma_start(out=at_view(exn), in_=exn_t[:])
            nc.scalar.dma_start(out=at_view(runn), in_=mc8[:])

    nc.compile()
    return nc


def _consts():
    iota = np.arange(M, dtype=np.float32).reshape(C, P)
    us16 = np.triu(np.ones((C, C), dtype=np.float32), 1)  # [k,i]=1 iff k<i
    on16 = np.ones((C, C), dtype=np.float32)
    id16 = np.eye(C, dtype=np.float32)
    id128 = np.eye(P, dtype=np.float32)
    return iota, us16, on16, id16, id128


def make_in_maps(x, run, acc_p, weighted_h, remainders, exit_, pad_h, p_w, p_b):
    iota, us16, on16, id16, id128 = _consts()
    pad128 = np.broadcast_to(
        np.asarray(pad_h, dtype=np.float32).reshape(1, H), (P, H)).copy()
    pw128 = np.broadcast_to(
        np.asarray(p_w, dtype=np.float32).reshape(1, H), (P, H)).copy()
    pb1 = np.asarray(p_b, dtype=np.float32).reshape(1)
    in_maps = []
    for b in range(NCORES):
        in_maps.append(
            {
                "x": np.ascontiguousarray(x[b], dtype=np.float32),
                "run8": np.ascontiguousarray(run[b]).astype(np.uint8),
                "accp": np.ascontiguousarray(acc_p[b]).reshape(M).astype(np.float32),
                "wh": np.ascontiguousarray(weighted_h[b], dtype=np.float32),
                "rem": np.ascontiguousarray(remainders[b]).reshape(M).astype(np.float32),
                "exi": np.ascontiguousarray(exit_[b]).reshape(M).astype(np.int32),
                "pad128": pad128,
                "pw128": pw128,
                "pb": pb1,
                "iota_c": iota,
                "us16": us16,
                "on16": on16,
                "id16": id16,
                "id128": id128,
            }
        )
    return in_maps


def kernel(x, run, acc_p, weighted_h, remainders, exit_, updates, pad_h, p_w, p_b,
           _want_results_obj=False, _trace=False):
    from concourse.bass_utils import run_bass_kernel_spmd

    x = np.asarray(x)
    run = np.asarray(run)
    acc_p = np.asarray(acc_p)
    weighted_h = np.asarray(weighted_h)
    remainders = np.asarray(remainders)
    exit_ = np.asarray(exit_)

    nc = _build(float(np.asarray(updates)) + 1.0)
    in_maps = make_in_maps(x, run, acc_p, weighted_h, remainders, exit_,
                           pad_h, p_w, p_b)
    res = run_bass_kernel_spmd(nc, in_maps, core_ids=list(range(NCORES)),
                               trace=_trace)

    h_packed = np.stack([res.results[b]["hp"] for b in range(NCORES)])
    whn = np.stack([res.results[b]["whn"] for b in range(NCORES)])
    accn = np.stack([res.results[b]["accn"] for b in range(NCORES)]).reshape(B, M, 1)
    remn = np.stack([res.results[b]["remn"] for b in range(NCORES)]).reshape(B, M, 1)
    runn = np.stack([res.results[b]["runn"] for b in range(NCORES)]).astype(bool)
    exn = np.stack([res.results[b]["exn"] for b in range(NCORES)]).reshape(B, M, 1)
    out = (h_packed, whn, accn, remn, runn, exn.astype(np.int32))
    if _want_results_obj:
        return out, res
    return out


# revision 15
# speedup vs baseline: 1.3241x; 1.2057x over previous
"""Adaptive Computation Time step — Trainium2 Bass kernel.

Pure data parallel over batch B=8 on 8 NeuronCores (one batch row per core).

Per-core layout conventions (M=2048 tokens, H=1024):
  - "AT" small-tensor layout: SBUF [16, 128], token m at [m // 128, m % 128].
    DRAM [2048] <-> AT is a contiguous 512B-per-partition DMA.
  - "A" big-tile layout (forced by dma_gather): token m at partition m % 128,
    free chunk m // 128.  H tile is [128, 16, 1024].
  - "G" index layout (forced by dma_gather/dma_scatter_add): int16 index for
    token m at [m % 16, m // 16] of a [128, 128] tile (only partitions 0-15
    are used; the rest must be zeroed for the bounds assert).

Algorithm per core:
  rank   = cumsum(run) - 1                    (scan + 16x16 tri matmul)
  h      = x[clamp(rank,0)]                   (dma_gather, 4 x 512 rows)
  p      = sigmoid(h @ p_w + p_b) * run
  mc     = (acc_p + p < 0.99) & run ; me = run & ~mc
  update = p*mc + (1-acc_p)*me
  weighted_h_new = h*update + weighted_h      (fused DVE op, streamed)
  c2     = cumsum(mc);  dest = mc ? c2-1 : count_new + (m - c2)   (bijection)
  H[~mc rows] = pad_h  (copy_predicated), then scatter-add H -> h_packed
  (h_packed output buffer starts zeroed, each row written exactly once)
"""

import functools
import sys

sys.path.insert(0, "/opt/trn_rl_repo")

import numpy as np

B, M, H = 8, 2048, 1024
P, C = 128, 16  # partitions / chunks; M == P * C
TH = 0.99
NCORES = 8


@functools.lru_cache(maxsize=4)
def _build(updates_plus1: float):
    import concourse.bass as bass
    import concourse.tile as tile
    from concourse import bacc, mybir

    f32 = mybir.dt.float32
    i32 = mybir.dt.int32
    i16 = mybir.dt.int16
    u8 = mybir.dt.uint8
    Alu = mybir.AluOpType
    Act = mybir.ActivationFunctionType

    nc = bacc.Bacc("TRN2", target_bir_lowering=False, debug=False)

    # ---- DRAM parameters (per-core shard shapes) ----
    x = nc.dram_tensor("x", [M, H], f32, kind="ExternalInput")
    run8 = nc.dram_tensor("run8", [M], u8, kind="ExternalInput")
    accp = nc.dram_tensor("accp", [M], f32, kind="ExternalInput")
    wh = nc.dram_tensor("wh", [M, H], f32, kind="ExternalInput")
    rem = nc.dram_tensor("rem", [M], f32, kind="ExternalInput")
    exi = nc.dram_tensor("exi", [M], i32, kind="ExternalInput")
    pad128_d = nc.dram_tensor("pad128", [P, H], f32, kind="ExternalInput")
    pw128_d = nc.dram_tensor("pw128", [P, H], f32, kind="ExternalInput")
    pb_d = nc.dram_tensor("pb", [1], f32, kind="ExternalInput")
    iota_d = nc.dram_tensor("iota_c", [C, P], f32, kind="ExternalInput")
    us16_d = nc.dram_tensor("us16", [C, C], f32, kind="ExternalInput")
    on16_d = nc.dram_tensor("on16", [C, C], f32, kind="ExternalInput")
    id16_d = nc.dram_tensor("id16", [C, C], f32, kind="ExternalInput")
    id128_d = nc.dram_tensor("id128", [P, P], f32, kind="ExternalInput")

    hp = nc.dram_tensor("hp", [M, H], f32, kind="ExternalOutput")
    whn = nc.dram_tensor("whn", [M, H], f32, kind="ExternalOutput")
    accn = nc.dram_tensor("accn", [M], f32, kind="ExternalOutput")
    remn = nc.dram_tensor("remn", [M], f32, kind="ExternalOutput")
    runn = nc.dram_tensor("runn", [M], u8, kind="ExternalOutput")
    exn = nc.dram_tensor("exn", [M], i32, kind="ExternalOutput")

    def at_view(dram):  # DRAM [2048] -> [16,128] AT view
        return dram.ap().rearrange("(t p) -> t p", p=P)

    with tile.TileContext(nc) as tc:
        with (
            tc.tile_pool(name="const", bufs=1) as cst,
            tc.tile_pool(name="small", bufs=1) as sm,
            tc.tile_pool(name="idx", bufs=1) as idxp,
            tc.tile_pool(name="big", bufs=1) as big,
            tc.tile_pool(name="whs", bufs=8) as whs,
            tc.tile_pool(name="outs", bufs=4) as outs,
            tc.tile_pool(name="scr", bufs=2) as scr,
            tc.tile_pool(name="ps", bufs=2, space="PSUM") as ps,
            tc.tile_pool(name="ps2", bufs=2, space="PSUM") as ps2,
        ):
            # critical path first: run8 + the two matrices the rank
            # computation needs, on the sync HWDGE queue
            run8_t = sm.tile([C, P], u8)
            nc.sync.dma_start(out=run8_t[:], in_=at_view(run8))
            pb1 = cst.tile([1, 1], f32)
            nc.sync.dma_start(out=pb1[:], in_=pb_d.ap()[None, :])
            pb128 = cst.tile([P, 1], f32)
            nc.gpsimd.partition_broadcast(pb128[:], pb1[:])
            # mlp-library warm-up: a dummy 16-row gather forces the gpsimd
            # library load (~9us) to happen now, off the real gather path.
            # (partition_broadcast alone would make the pass pick the attn
            # library and the real gather would pay a second load.)
            warm_idx = idxp.tile([P, 1], i16)
            nc.vector.memset(warm_idx[:], 0)
            warm_out = scr.tile([P, 1, H], f32, tag="scr")
            nc.gpsimd.dma_gather(
                out_ap=warm_out[:], in_ap=x.ap(), idxs_ap=warm_idx[:],
                num_idxs=16, num_idxs_reg=16, elem_size=H,
            )
            id16 = cst.tile([C, C], f32)
            nc.sync.dma_start(out=id16[:], in_=id16_d.ap())
            us16 = cst.tile([C, C], f32)
            nc.sync.dma_start(out=us16[:], in_=us16_d.ap())
            # non-critical constants + small inputs on the scalar queue
            id128 = cst.tile([P, P], f32)
            nc.scalar.dma_start(out=id128[:], in_=id128_d.ap())
            on16 = cst.tile([C, C], f32)
            nc.scalar.dma_start(out=on16[:], in_=on16_d.ap())
            iota = cst.tile([C, P], f32)
            nc.scalar.dma_start(out=iota[:], in_=iota_d.ap())
            pw128 = cst.tile([P, H], f32)
            nc.scalar.dma_start(out=pw128[:], in_=pw128_d.ap())
            pad128 = cst.tile([P, H], f32)
            nc.scalar.dma_start(out=pad128[:], in_=pad128_d.ap())
            accp_t = sm.tile([C, P], f32)
            nc.scalar.dma_start(out=accp_t[:], in_=at_view(accp))
            rem_t = sm.tile([C, P], f32)
            nc.scalar.dma_start(out=rem_t[:], in_=at_view(rem))
            exi_t = sm.tile([C, P], i32)
            nc.scalar.dma_start(out=exi_t[:], in_=at_view(exi))
            runf = sm.tile([C, P], f32)
            nc.vector.tensor_copy(out=runf[:], in_=run8_t[:])
            exif = sm.tile([C, P], f32)
            nc.vector.tensor_copy(out=exif[:], in_=exi_t[:])

            z16 = sm.tile([C, P], f32)
            nc.vector.memset(z16[:], 0.0)

            # ---------------- cumsum #1 (rank from run) ----------------
            s1 = sm.tile([C, P], f32)
            nc.vector.tensor_tensor_scan(
                out=s1[:], data0=runf[:], data1=z16[:], initial=0.0,
                op0=Alu.add, op1=Alu.add,
            )
            ex1 = ps2.tile([C, 1], f32, space="PSUM", tag="tmisc")
            nc.tensor.matmul(out=ex1[:], lhsT=us16[:], rhs=s1[:, P - 1 : P],
                             start=True, stop=True)
            # rank = (s1 + excl) - 1 ; gather idx = max(rank, 0)
            c1 = sm.tile([C, P], f32)
            nc.vector.tensor_scalar(
                out=c1[:], in0=s1[:], scalar1=ex1[:], scalar2=1.0,
                op0=Alu.add, op1=Alu.subtract,
            )
            gidxf = sm.tile([C, P], f32)
            nc.vector.tensor_single_scalar(
                out=gidxf[:], in_=c1[:], scalar=0.0, op=Alu.max
            )

            # ---------------- AT -> G conversion for gather idx ----------------
            gidx16 = idxp.tile([P, P], i16)
            gview = gidx16[0:C, :].rearrange("q (t a) -> q a t", a=8)
            for a in range(8):
                pt = ps.tile([C, C], f32, space="PSUM", tag="blkT")
                nc.tensor.transpose(pt[:], gidxf[:, a * C : (a + 1) * C], id16[:])
                nc.vector.tensor_copy(out=gview[:, a, :], in_=pt[:])
            # the gather ucode runs on 8 Q7 cores; each reads its own
            # 16-partition replica of the index block. HWDGE copies: they
            # complete while the Q7 stream is stalled on the library load.
            for r in range(1, 8):
                eng = nc.sync if r % 2 else nc.scalar
                eng.dma_start(out=gidx16[r * 16 : (r + 1) * 16, :],
                              in_=gidx16[0:16, :])

            # ---------------- big gather: H[token] = x[clamp(rank,0)] --------
            Ht = big.tile([P, C, H], f32)
            for g in range(8):
                nc.gpsimd.dma_gather(
                    out_ap=Ht[:, g * 2 : (g + 1) * 2, :],
                    in_ap=x.ap(),
                    idxs_ap=gidx16[:, g * 16 : (g + 1) * 16],
                    num_idxs=256,
                    num_idxs_reg=256,
                    elem_size=H,
                )

            # ---------------- weighted_h loads (independent) ----------------
            wh_tiles = []
            for t in range(C):
                wt = whs.tile([P, H], f32, tag="wht")
                nc.sync.dma_start(out=wt[:], in_=wh.ap()[t * P : (t + 1) * P, :])
                wh_tiles.append(wt)

            # ---------------- p-dot per chunk ----------------
            pdot = sm.tile([P, C], f32)
            for t in range(C):
                sc = scr.tile([P, H], f32, tag="scr")
                nc.vector.scalar_tensor_tensor(
                    out=sc[:], in0=Ht[:, t, :], scalar=1.0, in1=pw128[:],
                    op0=Alu.mult, op1=Alu.mult,
                    accum_out=pdot[:, t : t + 1],
                )

            sig_a = sm.tile([P, C], f32)
            nc.scalar.activation(sig_a[:], pdot[:], Act.Sigmoid,
                                 bias=pb128[:], scale=1.0)
            # transpose sigmoid result A -> AT
            pts = ps2.tile([C, P], f32, space="PSUM", tag="tmisc")
            nc.tensor.transpose(pts[:], sig_a[:], id128[:])
            sig = sm.tile([C, P], f32)
            nc.scalar.copy(out=sig[:], in_=pts[:])

            # ---------------- halting math (AT layout) ----------------
            p_at = sm.tile([C, P], f32)
            nc.vector.tensor_mul(out=p_at[:], in0=sig[:], in1=runf[:])
            tmp = sm.tile([C, P], f32)
            nc.vector.tensor_add(out=tmp[:], in0=accp_t[:], in1=p_at[:])
            mcf = sm.tile([C, P], f32)
            nc.vector.tensor_single_scalar(out=mcf[:], in_=tmp[:], scalar=TH,
                                           op=Alu.is_lt)
            mc = sm.tile([C, P], f32)
            nc.vector.tensor_mul(out=mc[:], in0=mcf[:], in1=runf[:])
            me = sm.tile([C, P], f32)
            nc.vector.tensor_sub(out=me[:], in0=runf[:], in1=mc[:])
            u1 = sm.tile([C, P], f32)
            nc.vector.tensor_mul(out=u1[:], in0=p_at[:], in1=mc[:])
            t2 = sm.tile([C, P], f32)
            nc.vector.tensor_mul(out=t2[:], in0=accp_t[:], in1=me[:])
            u2 = sm.tile([C, P], f32)
            nc.vector.tensor_sub(out=u2[:], in0=me[:], in1=t2[:])
            upd = sm.tile([C, P], f32)
            nc.vector.tensor_add(out=upd[:], in0=u1[:], in1=u2[:])
            accn_t = sm.tile([C, P], f32)
            nc.vector.tensor_add(out=accn_t[:], in0=accp_t[:], in1=u1[:])
            remn_t = sm.tile([C, P], f32)
            nc.vector.tensor_add(out=remn_t[:], in0=rem_t[:], in1=u2[:])
            exnf = sm.tile([C, P], f32)
            nc.vector.scalar_tensor_tensor(
                out=exnf[:], in0=me[:], scalar=updates_plus1, in1=exif[:],
                op0=Alu.mult, op1=Alu.add,
            )
            inv = sm.tile([C, P], f32)
            nc.vector.tensor_single_scalar(out=inv[:], in_=mc[:], scalar=0.0,
                                           op=Alu.is_equal)

            # ---------------- cumsum #2 (pack positions from mc) -------------
            s2 = sm.tile([C, P], f32)
            nc.vector.tensor_tensor_scan(
                out=s2[:], data0=mc[:], data1=z16[:], initial=0.0,
                op0=Alu.add, op1=Alu.add,
            )
            ex2 = ps2.tile([C, 1], f32, space="PSUM", tag="tmisc")
            nc.tensor.matmul(out=ex2[:], lhsT=us16[:], rhs=s2[:, P - 1 : P],
                             start=True, stop=True)
            cnt = ps2.tile([C, 1], f32, space="PSUM", tag="tmisc")
            nc.tensor.matmul(out=cnt[:], lhsT=on16[:], rhs=s2[:, P - 1 : P],
                             start=True, stop=True)
            c2 = sm.tile([C, P], f32)
            nc.vector.tensor_scalar(
                out=c2[:], in0=s2[:], scalar1=ex2[:], scalar2=1.0,
                op0=Alu.add, op1=Alu.subtract,
            )
            # tail slot = (iota - c2) + count_new - 1
            tl0 = sm.tile([C, P], f32)
            nc.vector.tensor_sub(out=tl0[:], in0=iota[:], in1=c2[:])
            dest = sm.tile([C, P], f32)
            nc.vector.tensor_scalar(
                out=dest[:], in0=tl0[:], scalar1=cnt[:], scalar2=1.0,
                op0=Alu.add, op1=Alu.subtract,
            )
            mc8 = sm.tile([C, P], u8)
            nc.vector.tensor_copy(out=mc8[:], in_=mc[:])
            nc.vector.copy_predicated(out=dest[:], mask=mc8[:], data=c2[:])

            # scatter dest idx to A layout [128,16] int32 (for indirect DMA)
            ptd = ps.tile([P, C], f32, space="PSUM", tag="destT")
            nc.tensor.transpose(ptd[:], dest[:], id16[:])
            didx_a = sm.tile([P, C], i32)
            nc.vector.tensor_copy(out=didx_a[:], in_=ptd[:])

            # ---------------- update / pad-mask to A layout ----------------
            ptu = ps2.tile([P, C], f32, space="PSUM", tag="tmisc")
            nc.tensor.transpose(ptu[:], upd[:], id16[:])
            upd_a = sm.tile([P, C], f32)
            nc.scalar.copy(out=upd_a[:], in_=ptu[:])
            pti = ps2.tile([P, C], f32, space="PSUM", tag="tmisc")
            nc.tensor.transpose(pti[:], inv[:], id16[:])
            inv_a = sm.tile([P, C], u8)
            nc.vector.tensor_copy(out=inv_a[:], in_=pti[:])

            # ---------------- FMA + stores, pad-predicate, scatter -----------
            for t in range(C):
                ot = outs.tile([P, H], f32, tag="outt")
                nc.vector.scalar_tensor_tensor(
                    out=ot[:], in0=Ht[:, t, :], scalar=upd_a[:, t : t + 1],
                    in1=wh_tiles[t][:], op0=Alu.mult, op1=Alu.add,
                )
                nc.scalar.dma_start(out=whn.ap()[t * P : (t + 1) * P, :], in_=ot[:])
                nc.vector.copy_predicated(
                    out=Ht[:, t, :],
                    mask=inv_a[:, t : t + 1].to_broadcast([P, H]),
                    data=pad128[:],
                )
                # pack: plain scatter (no RMW, unlike dma_scatter_add) of the
                # chunk's 128 rows to their bijective destinations
                nc.gpsimd.indirect_dma_start(
                    out=hp.ap(),
                    out_offset=bass.IndirectOffsetOnAxis(
                        ap=didx_a[:, t : t + 1], axis=0),
                    in_=Ht[:, t, :],
                    in_offset=None,
                )
                # all calls' out_ap covers all of hp, but destination rows
                # are disjoint by construction (dest is a bijection) — drop
                # the spurious WAW chain so the scatters pipeline; the
                # kernel-tail gpsimd drain still awaits the SWDGE queues
                tc.dep_state.clear_tensor_accesses(hp.name)

            # ---------------- small outputs ----------------
            nc.scalar.dma_start(out=at_view(accn), in_=accn_t[:])
            nc.scalar.dma_start(out=at_view(remn), in_=remn_t[:])
            exn_t = sm.tile([C, P], i32)
            nc.vector.tensor_copy(out=exn_t[:], in_=exnf[:])
            nc.scalar.dma_start(out=at_view(exn), in_=exn_t[:])
            nc.scalar.dma_start(out=at_view(runn), in_=mc8[:])

    nc.compile()
    return nc


def _consts():
    iota = np.arange(M, dtype=np.float32).reshape(C, P)
    us16 = np.triu(np.ones((C, C), dtype=np.float32), 1)  # [k,i]=1 iff k<i
    on16 = np.ones((C, C), dtype=np.float32)
    id16 = np.eye(C, dtype=np.float32)
    id128 = np.eye(P, dtype=np.float32)
    return iota, us16, on16, id16, id128


def make_in_maps(x, run, acc_p, weighted_h, remainders, exit_, pad_h, p_w, p_b):
    iota, us16, on16, id16, id128 = _consts()
    pad128 = np.broadcast_to(
        np.asarray(pad_h, dtype=np.float32).reshape(1, H), (P, H)).copy()
    pw128 = np.broadcast_to(
        np.asarray(p_w, dtype=np.float32).reshape(1, H), (P, H)).copy()
    pb1 = np.asarray(p_b, dtype=np.float32).reshape(1)
    in_maps = []
    for b in range(NCORES):
        in_maps.append(
            {
                "x": np.ascontiguousarray(x[b], dtype=np.float32),
                "run8": np.ascontiguousarray(run[b]).astype(np.uint8),
                "accp": np.ascontiguousarray(acc_p[b]).reshape(M).astype(np.float32),
                "wh": np.ascontiguousarray(weighted_h[b], dtype=np.float32),
                "rem": np.ascontiguousarray(remainders[b]).reshape(M).astype(np.float32),
                "exi": np.ascontiguousarray(exit_[b]).reshape(M).astype(np.int32),
                "pad128": pad128,
                "pw128": pw128,
                "pb": pb1,
                "iota_c": iota,
                "us16": us16,
                "on16": on16,
                "id16": id16,
                "id128": id128,
            }
        )
    return in_maps


def kernel(x, run, acc_p, weighted_h, remainders, exit_, updates, pad_h, p_w, p_b,
           _want_results_obj=False, _trace=False):
    from concourse.bass_utils import run_bass_kernel_spmd

    x = np.asarray(x)
    run = np.asarray(run)
    acc_p = np.asarray(acc_p)
    weighted_h = np.asarray(weighted_h)
    remainders = np.asarray(remainders)
    exit_ = np.asarray(exit_)

    nc = _build(float(np.asarray(updates)) + 1.0)
    in_maps = make_in_maps(x, run, acc_p, weighted_h, remainders, exit_,
                           pad_h, p_w, p_b)
    res = run_bass_kernel_spmd(nc, in_maps, core_ids=list(range(NCORES)),
                               trace=_trace)

    h_packed = np.stack([res.results[b]["hp"] for b in range(NCORES)])
    whn = np.stack([res.results[b]["whn"] for b in range(NCORES)])
    accn = np.stack([res.results[b]["accn"] for b in range(NCORES)]).reshape(B, M, 1)
    remn = np.stack([res.results[b]["remn"] for b in range(NCORES)]).reshape(B, M, 1)
    runn = np.stack([res.results[b]["runn"] for b in range(NCORES)]).astype(bool)
    exn = np.stack([res.results[b]["exn"] for b in range(NCORES)]).reshape(B, M, 1)
    out = (h_packed, whn, accn, remn, runn, exn.astype(np.int32))
    if _want_results_obj:
        return out, res
    return out
